# revision 11
# baseline (speedup 1.0000x reference)
"""KV page-cache scatter update on 8 Trainium2 NeuronCores — in-place.

Semantics (matches the reference):
    kv_ev = interleave(new_k, new_v)          # [T, 2H, D], head axis k0,v0,k1,v1,...
    for i in range(K):
        kv_pages[t_pages[i], t_slots[i]] = kv_ev[i]
    return kv_pages

Key idea: the output equals the input except for K scattered 8KB rows, so
the 268MB bulk copy never needs to touch the device engines.  The axon
PJRT path for Bass kernels binds donated operands by name to the NEFF's
ExternalOutput tensors (bass2jax.run_bass_via_pjrt donates zero buffers
this way, and kernels that don't write every output element rely on it).
We donate the kv_pages shard ITSELF as the output buffer, so the device
program is only the scatter: stage the compacted update rows into SBUF
and indirect-DMA them into the donated page buffer.  Per-core device
traffic drops from ~70MB (copy in+out) to ~3MB (update rows in+out),
which sits at the SDMA-fabric roofline for this size.

Sharding: pages are assigned to 16 buckets — (core, shard-half) pairs,
128 pages each — by a greedy balance on per-page update counts, so every
bucket carries an (almost) equal number of update rows and the SPMD
padding (all cores stage the same nph rows) is minimal: for the K=1536
regime the balance is exact, nph=96 vs 112 for fixed contiguous page
blocks (~14%% less staged+scattered traffic).  The host gathers each
bucket's pages into the donated buffers and inverse-scatters the device
output back to the original page order; both are page-granular (131KB)
host memcpys, the same volume the unbalanced layout needs anyway.

The output shard is split into TWO donated DRAM tensors (halves) so the
two indirect scatters have no false dependency and run concurrently;
per-half staging on separate HWDGE rings lets half A's scatter start
while half B's rows are still streaming into SBUF.  Destination indices
for both halves are staged by ONE strided SWDGE DMA as a [nph, 2] tile,
keeping the tiny-descriptor spray off the HWDGE rings.  Destinations are
unique (page,slot) pairs, sorted for HBM write locality; padding
duplicates the last valid row (identical concurrent writes are benign).
"""

import numpy as np
import jax
import jax.numpy as jnp
# same import bass2jax uses; the new jax.shard_map renamed check_rep->check_vma
from jax.experimental.shard_map import shard_map
from jax.sharding import Mesh, NamedSharding, PartitionSpec

from concourse import bacc, bass, bass2jax, mybir, tile

# Problem geometry (hardcoded per contract).
P, S, HH, D = 2048, 16, 16, 128   # pages, slots/page, 2*kv_heads, head_dim
T = 2048                          # new tokens
NCORES = 8
PC = P // NCORES                  # pages per core
RC = PC * S                       # flat rows per core (4096)
H = 2                             # output split (independent scatters)
HR = RC // H                      # rows per half (2048)
RD = HH * D                       # row width in f32 (2048 = 8KB)
NBK = NCORES * H                  # 16 page buckets
CAP = P // NBK                    # 128 pages per bucket

_PROGRAM_CACHE: dict[tuple, object] = {}
_RUNNER_CACHE: dict[int, tuple] = {}
_LAST_INS = None                  # stashed for test.py's bench
_LAST_NPH = None


def _build_program(nph: int, reps: int = 1):
    """Scatter-only Bass program: stage 2*nph update rows + dest indices
    into SBUF, indirect-DMA each half's rows into its donated output half.

    reps > 1 repeats the body inside one NEFF (reps serialized by Tile's
    dependency tracking on the output halves) so a slope over rep counts
    cancels dispatch overhead — used by the bench only.
    """
    key = (nph, reps)
    if key in _PROGRAM_CACHE:
        return _PROGRAM_CACHE[key]
    nc = bacc.Bacc("TRN2", target_bir_lowering=False, debug=False)
    upd = nc.dram_tensor("upd", [H * nph, RD], mybir.dt.float32,
                         kind="ExternalInput")
    # dest laid out [nph, H] (row i = the i-th index of every half) so the
    # stage below is a contiguous read: one 8B descriptor per partition
    # instead of H strided 4B ones
    dest = nc.dram_tensor("dest", [nph, H], mybir.dt.int32,
                          kind="ExternalInput")
    outs = [nc.dram_tensor(f"kv_out{h}", [HR, RD], mybir.dt.float32,
                           kind="ExternalOutput") for h in range(H)]
    # SBUF tiles cap at 128 partitions; for nph > 128 (not reachable for
    # K <= 2048 with balanced buckets, but kept for safety) split each
    # half into <=128-row sub-blocks.
    nb = -(-nph // 128)
    blocks = [(b * 128, min(nph, (b + 1) * 128)) for b in range(nb)]
    with tile.TileContext(nc) as tc:
        with tc.tile_pool(name="sbuf", bufs=max(2, 2 * H * nb)) as pool:
            for _rep in range(reps):
                # dest indices first (tiny; they gate scatter emission).
                # One strided SWDGE DMA per block stages both halves'
                # indices as a [blk, H] tile — keeps the tiny-descriptor
                # spray off the HWDGE rings that stream the update rows.
                dtiles = []
                for lo, hi in blocks:
                    dtile = pool.tile([hi - lo, H], mybir.dt.int32)
                    nc.gpsimd.dma_start(out=dtile[:], in_=dest[lo:hi, :])
                    dtiles.append(dtile)
                # update rows: each half's stage split across BOTH HWDGE
                # rings so staging latency halves and the scatters start
                # sooner (measured ~0.5us better than ring-per-half)
                utiles = []
                for h in range(H):
                    for lo, hi in blocks:
                        utile = pool.tile([hi - lo, RD], mybir.dt.float32)
                        utiles.append(utile)
                for h in range(H):
                    for b, (lo, hi) in enumerate(blocks):
                        ut = utiles[h * nb + b]
                        blk = hi - lo
                        if blk >= 2:
                            half = blk // 2
                            nc.sync.dma_start(
                                out=ut[:half, :],
                                in_=upd[h * nph + lo:h * nph + lo + half, :])
                            nc.scalar.dma_start(
                                out=ut[half:, :],
                                in_=upd[h * nph + lo + half:h * nph + hi, :])
                        else:
                            nc.sync.dma_start(
                                out=ut[:],
                                in_=upd[h * nph + lo:h * nph + hi, :])
                for h in range(H):
                    for b in range(nb):
                        nc.gpsimd.indirect_dma_start(
                            out=outs[h][:],
                            out_offset=bass.IndirectOffsetOnAxis(
                                ap=dtiles[b][:, h:h + 1], axis=0),
                            in_=utiles[h * nb + b][:],
                            in_offset=None,
                        )
    nc.compile()
    _PROGRAM_CACHE[key] = nc
    return nc


def _make_runner(nc, donate: bool = True):
    """Reusable jitted runner for a compiled Bass program on the 8 axon
    cores — the same lowering bass2jax.run_bass_via_pjrt performs, but
    built once and reused, with the donated output-named operands under
    caller control (we pass the kv shard instead of zeros)."""
    bass2jax.install_neuronx_cc_hook()
    partition_name = (nc.partition_id_tensor.name
                      if nc.partition_id_tensor else None)
    in_names, out_names, out_avals = [], [], []
    for alloc in nc.m.functions[0].allocations:
        if not isinstance(alloc, mybir.MemoryLocationSet):
            continue
        name = alloc.memorylocations[0].name
        if alloc.kind == "ExternalInput":
            if name != partition_name:
                in_names.append(name)
        elif alloc.kind == "ExternalOutput":
            out_names.append(name)
            out_avals.append(jax.core.ShapedArray(
                tuple(alloc.tensor_shape), mybir.dt.np(alloc.dtype)))
    n_params = len(in_names)
    n_outs = len(out_names)
    all_in_names = list(in_names) + list(out_names)
    if partition_name is not None:
        all_in_names.append(partition_name)

    def _body(*args):
        operands = list(args)
        if partition_name is not None:
            operands.append(bass2jax.partition_id_tensor())
        return tuple(bass2jax._bass_exec_p.bind(
            *operands,
            out_avals=tuple(out_avals),
            in_names=tuple(all_in_names),
            out_names=tuple(out_names),
            lowering_input_output_aliases=(),
            sim_require_finite=True,
            sim_require_nnan=True,
            nc=nc,
        ))

    devices = jax.devices()[:NCORES]
    mesh = Mesh(np.asarray(devices), ("core",))
    kw = dict(keep_unused=True)
    if donate:
        kw["donate_argnums"] = tuple(range(n_params, n_params + n_outs))
    fn = jax.jit(
        shard_map(_body, mesh=mesh,
                  in_specs=(PartitionSpec("core"),) * (n_params + n_outs),
                  out_specs=(PartitionSpec("core"),) * n_outs,
                  check_rep=False),
        **kw,
    )
    return fn, mesh, in_names, out_names, out_avals


def _balance(counts):
    """Greedy first-fit-decreasing: assign pages to NBK buckets (capacity
    CAP pages) minimizing the max per-bucket update-row count.  For the
    uniform K=1536 regime the result is an exact 96-per-bucket balance."""
    order = np.argsort(-counts, kind="stable")
    loads = np.zeros(NBK, dtype=np.int64)
    sizes = np.zeros(NBK, dtype=np.int64)
    buckets = np.empty((NBK, CAP), dtype=np.int64)
    for p in order:
        best = -1
        for i in range(NBK):
            if sizes[i] < CAP and (best < 0 or loads[i] < loads[best]):
                best = i
        buckets[best, sizes[best]] = p
        sizes[best] += 1
        loads[best] += counts[p]
    return buckets, int(loads.max())


def kernel(kv_pages, t_pages, t_slots, new_k, new_v, K):
    kv_pages = np.ascontiguousarray(np.asarray(kv_pages), dtype=np.float32)
    t_pages = np.asarray(t_pages)
    t_slots = np.asarray(t_slots)
    new_k = np.asarray(new_k)
    new_v = np.asarray(new_v)
    k_valid = max(0, min(int(np.asarray(K)), new_k.shape[0]))

    out_dtype = np.asarray(kv_pages).dtype
    Tn, Hn, Dn = new_k.shape

    # interleave K/V along the head axis: [T, 2H, D] -> flat [T, RD]
    kv_ev = np.empty((Tn, 2 * Hn, Dn), dtype=np.float32)
    kv_ev[:, 0::2, :] = new_k
    kv_ev[:, 1::2, :] = new_v
    kv_ev = kv_ev.reshape(Tn, 2 * Hn * Dn)

    pages = t_pages[:k_valid].astype(np.int64)
    slots = t_slots[:k_valid].astype(np.int64)
    counts = np.bincount(pages, minlength=P)
    buckets, maxn = _balance(counts)
    bucket_of = np.empty(P, dtype=np.int64)
    pos_in_bucket = np.empty(P, dtype=np.int64)
    flat_idx = buckets.reshape(-1)
    bucket_of[flat_idx] = np.repeat(np.arange(NBK), CAP)
    pos_in_bucket[flat_idx] = np.tile(np.arange(CAP), NBK)
    nph = max(16, -(-maxn // 4) * 4)

    kv_paged = kv_pages.reshape(P, S * RD)
    tok_bucket = bucket_of[pages]
    tok_rel = (pos_in_bucket[pages] * S + slots).astype(np.int32)

    upds, dests = [], []
    for c in range(NCORES):
        u = np.empty((H * nph, RD), dtype=np.float32)
        d = np.empty((nph, H), dtype=np.int32)
        for h in range(H):
            k = c * H + h
            gi = np.nonzero(tok_bucket == k)[0]
            rel = tok_rel[gi]
            o = np.argsort(rel, kind="stable")
            gi, rel = gi[o], rel[o]
            n = len(gi)
            off = h * nph
            if n > 0:
                u[off:off + n] = kv_ev[gi]
                d[:n, h] = rel
                u[off + n:off + nph] = u[off + n - 1]
                d[n:, h] = d[n - 1, h]
            else:
                # no updates for this bucket: rewrite its row 0 with the
                # original data (identical concurrent writes are benign)
                u[off:off + nph] = kv_paged[buckets[k, 0], :RD]
                d[:, h] = 0
        upds.append(u)
        dests.append(d)
    ins = {"upd": np.concatenate(upds, 0), "dest": np.concatenate(dests, 0)}

    global _LAST_INS, _LAST_NPH
    _LAST_INS, _LAST_NPH = ins, nph

    if nph not in _RUNNER_CACHE:
        _RUNNER_CACHE[nph] = _make_runner(_build_program(nph, reps=1))
    fn, mesh, in_names, out_names, out_avals = _RUNNER_CACHE[nph]
    sh = NamedSharding(mesh, PartitionSpec("core"))

    din = [jax.device_put(ins[name], sh) for name in in_names]
    # donated output buffers: half h = bucket (c, h) pages, cores stacked
    half_pages = [
        buckets.reshape(NCORES, H, CAP)[:, h].reshape(-1) for h in range(H)
    ]
    dkv = [
        jax.device_put(
            np.ascontiguousarray(kv_paged[hp]).reshape(NCORES * HR, RD), sh)
        for hp in half_pages
    ]
    outs = fn(*din, *dkv)

    res = np.empty((P, S * RD), dtype=np.float32)
    for h, o in enumerate(outs):
        res[half_pages[h]] = np.asarray(o).reshape(P // H, S * RD)
    return res.reshape(P, S, HH, D).astype(out_dtype, copy=False)
